# revision 11
# baseline (speedup 1.0000x reference)
"""Trainium2 Bass kernel for nn_Codebook (vq_codebook).

Pipeline per core (16 of 128 images, data-parallel over 8 cores):
  1. Precompute (on device, once per core):
     - CBT_k [128, 1024]: codebook transposed into (c,ph)x{pw,pw+1} pair layout
     - cbsq broadcast [128, 1024]
     - T[b, :] = sum_j gauss(b - j) * codebook[j, :]  (banded Toeplitz matmul)
       stored to DRAM scratch for row-gather.
  2. Per image: load x twice into SBUF partitions 64-127 (offset 0) and
     0-63 (shifted +1 element) so a single strided AP provides the
     transposed patch matrix with full 128-deep contraction.
  3. scores m = (2*G - rowsq) - cbsq  (exact negation of the reference's
     f32-rounded d2, so argmax matches jnp.argmin bit-for-bit).
  4. DVE max/max_index -> bmu; indirect DMA gathers T[bmu] rows.
  5. Unpatchify via 4 PE transposes per row-tile + strided ACT copies;
     contiguous 256B-run DMA out.
"""
import math
import sys

if "/opt/trn_rl_repo" not in sys.path:
    sys.path.insert(0, "/opt/trn_rl_repo")

import numpy as np

import concourse.bass as bass
import concourse.mybir as mybir
from concourse.bass import IndirectOffsetOnAxis
from concourse.masks import make_identity
from concourse.tile import TileContext

# ---------------------------------------------------------------------------
# Patch: this walrus build rejects >1 sem wait on the tail Drain; spread the
# waits across single-wait SP nops instead.
import concourse.tile as _tile_mod
from concourse.vector_clock import ScopedClock as _ScopedClock


def _patched_drain_and_barrier(self, tick_clock, wait_clock):
    nc = self.nc
    drain_inst = nc.sync.drain()
    wait_clock.add_sem_waits(
        drain_inst.ins, _ScopedClock({None: tick_clock.global_clock})
    )
    si = drain_inst.ins.sync_info
    waits = list(si.on_wait) if si is not None else []
    if len(waits) > 1:
        si.on_wait = waits[:1]
        for w in waits[1:]:
            nop = nc.sync.nop(nofuse=True)
            nop.ins.sync_info = mybir.SyncInfo(on_wait=[w], on_update=[])

    nc.all_engine_barrier()
    assert self.sems is not None
    popped = nc._tile_sem_poison_stack.pop()
    assert popped is self._sem_poison
    nc.clear_and_free_semaphores(list(self.sems.allocated().values()))
    nc.all_engine_barrier()


_tile_mod.TileContext._drain_and_barrier = _patched_drain_and_barrier

# Generalized: any instruction may carry at most MAX_WAITS sem waits in this
# walrus build. Hoist extras onto same-engine NoOps committed just before
# (engines execute in order, so an earlier wait is equivalent).
MAX_WAITS = 1
_orig_commit = _tile_mod.TileContext._commit_instruction
_waitsplit_id = [0]


def _patched_commit(self, inst, lazy_reg_writes=True):
    si = inst.sync_info
    if si is not None:
        waits = list(si.on_wait)
        if len(waits) > MAX_WAITS:
            keep = waits[:MAX_WAITS - 1] if MAX_WAITS > 1 else []
            extra = waits[len(keep):]
            si.on_wait = keep + extra[-1:]
            extra = extra[:-1]
            for i in range(0, len(extra), MAX_WAITS):
                _waitsplit_id[0] += 1
                nop = mybir.InstNoOp(
                    name=f"I-waitsplit-{_waitsplit_id[0]}", ins=[], outs=[]
                )
                nop.engine = inst.engine
                nop.sync_info = mybir.SyncInfo(
                    on_wait=extra[i:i + MAX_WAITS], on_update=[]
                )
                self._add_instruction(nop)
    return _orig_commit(self, inst, lazy_reg_writes)


_tile_mod.TileContext._commit_instruction = _patched_commit
# ---------------------------------------------------------------------------

F32 = mybir.dt.float32
U32 = mybir.dt.uint32
AX = mybir.AxisListType
ALU = mybir.AluOpType
ACTF = mybir.ActivationFunctionType

N_CORES = 8
NI = 16          # images per core
K = 1024         # codebook entries
D = 256          # embedding dim
NEG_INV_2VAR = -1.0 / (2.0 * (-(256.0 / (2.0 * math.log(0.1)))))


def build_kernel():
    nc = bass.Bass()
    x = nc.dram_tensor("x", [NI, 16, 64, 64], F32, kind="ExternalInput")
    cb = nc.dram_tensor("codebook", [K, D], F32, kind="ExternalInput")
    out = nc.dram_tensor("out", [NI, 16, 64, 64], F32, kind="ExternalOutput")
    t_dram = nc.dram_tensor("t_scratch", [K, D], F32, kind="Internal")

    with TileContext(nc) as tc:
        with (
            tc.tile_pool(name="const", bufs=1) as cpool,
            tc.tile_pool(name="x2", bufs=2) as xpool,
            tc.tile_pool(name="scores", bufs=2) as spool,
            tc.tile_pool(name="small", bufs=4) as smpool,
            tc.tile_pool(name="q", bufs=3) as qpool,
            tc.tile_pool(name="outsb", bufs=2) as opool,
            tc.tile_pool(name="pg", bufs=4, space="PSUM") as pg,
            tc.tile_pool(name="pgr", bufs=2, space="PSUM") as pgr,
            tc.tile_pool(name="pu", bufs=2, space="PSUM") as pu,
        ):
            # ---------------- constants / precompute ----------------
            ident = cpool.tile([128, 128], F32)
            make_identity(nc, ident[:])
            ones_col = cpool.tile([128, 1], F32)
            nc.vector.memset(ones_col[:], 1.0)
            ones_row = cpool.tile([1, 128], F32)
            nc.vector.memset(ones_row[:], 1.0)

            # W band table [128, 384]: W[p, u] = gauss(u - 128 - p)
            wtab = cpool.tile([128, 384], F32)
            nc.gpsimd.iota(
                wtab[:], pattern=[[1, 384]], base=-128, channel_multiplier=-1,
                allow_small_or_imprecise_dtypes=True,
            )
            wsq = cpool.tile([128, 384], F32)
            nc.scalar.activation(wsq[:], wtab[:], ACTF.Square)
            nc.scalar.activation(wtab[:], wsq[:], ACTF.Exp, scale=NEG_INV_2VAR)

            # codebook natural chunks: cb_all[p, jc*256+d] = cb[jc*128+p, d]
            cb_all = cpool.tile([128, 8 * D], F32)
            nc.sync.dma_start(
                out=cb_all[:].rearrange("p (jc d) -> p jc d", jc=8),
                in_=cb[:].rearrange("(jc p) d -> p jc d", p=128),
            )

            # CBT_k [128, 1024]: p<64 -> cb[j, 4a+k]; p>=64 -> cb[j, 4a+k+1]
            cbt0 = cpool.tile([128, K], F32, tag="cbt0")
            cbt2 = cpool.tile([128, K], F32, tag="cbt2")
            cbt = {0: cbt0, 2: cbt2}
            stage0 = cpool.tile([64, K], F32, tag="stage0")
            stage2 = cpool.tile([64, K], F32, tag="stage2")
            stage = {0: stage0, 2: stage2}
            cb_r = cb_all[:].rearrange("p (jc a pw) -> p jc a pw", jc=8, pw=4)
            for jc in range(8):
                for pw in range(4):
                    k = 0 if pw < 2 else 2
                    # pw even -> cbt_k lower half; pw odd -> staged, DMA-shifted
                    tgt = cbt[k] if pw % 2 == 0 else stage[k]
                    tp = pu.tile([64, 128], F32, tag="pu")
                    # in_: [128 j, 64 a] strided slice (offset pw, stride 4)
                    src = cb_r[:, jc, :, pw]
                    nc.tensor.transpose(out=tp[:], in_=src, identity=ident[:])
                    nc.scalar.activation(
                        tgt[0:64, jc * 128:(jc + 1) * 128], tp[:], ACTF.Copy
                    )
            for k in (0, 2):
                nc.sync.dma_start(out=cbt[k][64:128, :], in_=stage[k][0:64, :])

            # cbsq row then broadcast [128, 1024]
            cbsq_bcast = cpool.tile([128, K], F32)
            sqtmp = spool.tile([128, K], F32, tag="scores")
            pb = pgr.tile([1, 512], F32, tag="pgr")
            pb2 = pgr.tile([1, 512], F32, tag="pgr")
            pbs = (pb, pb2)  # noqa
            for ki, k in enumerate((0, 2)):
                nc.vector.tensor_tensor(sqtmp[:], cbt[k][:], cbt[k][:], ALU.mult)
                for h in range(2):
                    nc.tensor.matmul(
                        pbs[h][:], ones_col[:], sqtmp[:, h * 512:(h + 1) * 512],
                        start=(ki == 0), stop=(ki == 1),
                    )
            cbsq_row = smpool.tile([1, K], F32, tag="cbsqrow")
            for h in range(2):
                nc.scalar.activation(
                    cbsq_row[0:1, h * 512:(h + 1) * 512], pbs[h][:], ACTF.Copy
                )
            for h in range(2):
                pc = pg.tile([128, 512], F32, tag="pg")
                nc.tensor.matmul(
                    pc[:], ones_row[:], cbsq_row[0:1, h * 512:(h + 1) * 512],
                    start=True, stop=True,
                )
                nc.scalar.activation(
                    cbsq_bcast[:, h * 512:(h + 1) * 512], pc[:], ACTF.Copy
                )

            # T table: T[bc*128+p, :] = sum_j gauss(b-j) cb[j, :]
            t_write_insts = []
            for bc in range(8):
                pt = pg.tile([128, 512], F32, tag="pg")
                deltas = [d_ for d_ in (-1, 0, 1) if 0 <= bc + d_ < 8]
                for i, d_ in enumerate(deltas):
                    off = 128 * (1 - d_)
                    jc = bc + d_
                    nc.tensor.matmul(
                        pt[:, 0:D],
                        wtab[:, off:off + 128],
                        cb_all[:, jc * D:(jc + 1) * D],
                        start=(i == 0), stop=(i == len(deltas) - 1),
                    )
                t_sb = qpool.tile([128, D], F32, tag="q")
                nc.scalar.activation(t_sb[:], pt[:, 0:D], ACTF.Copy)
                wi = nc.sync.dma_start(
                    out=t_dram[bc * 128:(bc + 1) * 128, :], in_=t_sb[:]
                )
                t_write_insts.append(wi.ins)

            # ---------------- main loop over images ----------------
            for n in range(NI):
                x2 = xpool.tile([128, 1028], F32)
                for ph in range(4):
                    # partitions 64 + 4c + ph  <-  x[n, :, ph::4, :]
                    nc.sync.dma_start(
                        out=x2[64 + ph:128:4, 0:1024],
                        in_=x[n][:, ph::4, :],
                    )
                nc.sync.dma_start(out=x2[0:64, 1:1025], in_=x2[64:128, 0:1024])

                out_sb = opool.tile([64, 1024], F32)

                for t in range(2):
                    pg0 = pg.tile([128, 512], F32, tag="pg")
                    pg1 = pg.tile([128, 512], F32, tag="pg")
                    pgram = pgr.tile([128, 128], F32, tag="pgr")
                    for ki, k in enumerate((0, 2)):
                        o = 1 + k + 512 * t
                        lhsT = x2[:, o:o + 512].rearrange(
                            "p (hp wp pw) -> p hp wp pw", wp=16, pw=4
                        )[:, :, :, 0]
                        st, sp = (ki == 0), (ki == 1)
                        nc.tensor.matmul(pg0[:], lhsT, cbt[k][:, 0:512],
                                         start=st, stop=sp)
                        nc.tensor.matmul(pg1[:], lhsT, cbt[k][:, 512:1024],
                                         start=st, stop=sp)
                        nc.tensor.matmul(pgram[:], lhsT, lhsT,
                                         start=st, stop=sp)
                    # rowsq = diag(gram)
                    diag = smpool.tile([128, 128], F32, tag="diag")
                    nc.vector.tensor_tensor(diag[:], pgram[:], ident[:], ALU.mult)
                    rowsq = smpool.tile([128, 1], F32, tag="rowsq")
                    nc.vector.tensor_reduce(rowsq[:], diag[:], AX.X, ALU.add)
                    nrowsq = smpool.tile([128, 1], F32, tag="nrowsq")
                    nc.vector.tensor_scalar(
                        nrowsq[:], rowsq[:], -1.0, None, op0=ALU.mult
                    )
                    # m = (2G - rowsq) - cbsq  == -(reference d2), same rounding
                    sc = spool.tile([128, K], F32, tag="scores")
                    nc.scalar.activation(
                        sc[:, 0:512], pg0[:], ACTF.Identity,
                        bias=nrowsq[:, 0:1], scale=2.0,
                    )
                    nc.scalar.activation(
                        sc[:, 512:1024], pg1[:], ACTF.Identity,
                        bias=nrowsq[:, 0:1], scale=2.0,
                    )
                    nc.vector.tensor_tensor(sc[:], sc[:], cbsq_bcast[:], ALU.subtract)
                    mx8 = smpool.tile([128, 8], F32, tag="mx8")
                    idx8 = smpool.tile([128, 8], U32, tag="idx8")
                    nc.vector.max(mx8[:], sc[:])
                    nc.vector.max_index(idx8[:], mx8[:], sc[:])

                    q = qpool.tile([128, D], F32, tag="q")
                    gi = nc.gpsimd.indirect_dma_start(
                        out=q[:],
                        out_offset=None,
                        in_=t_dram[:],
                        in_offset=IndirectOffsetOnAxis(ap=idx8[:, 0:1], axis=0),
                    )
                    for twi in t_write_insts:
                        _tile_mod.add_dep_helper(
                            gi.ins, twi, reason="gather waits for T table"
                        )

                    # unpatchify: 4 pw-slice transposes + strided copies
                    q_r = q[:].rearrange("p (a pw) -> p a pw", pw=4)
                    o_r = out_sb[:].rearrange(
                        "p (hp wp pw) -> p hp wp pw", wp=16, pw=4
                    )
                    for pw in range(4):
                        put = pu.tile([64, 128], F32, tag="pu")
                        nc.tensor.transpose(
                            out=put[:], in_=q_r[:, :, pw], identity=ident[:],
                        )
                        dst = o_r[0:64, 8 * t:8 * (t + 1), :, pw]
                        nc.scalar.activation(dst, put[:], ACTF.Copy)

                for ph in range(4):
                    nc.sync.dma_start(
                        out=out[n][:, ph::4, :],
                        in_=out_sb[ph:64:4, :],
                    )

    return nc


_NC_CACHE = None


def _get_nc():
    global _NC_CACHE
    if _NC_CACHE is None:
        _NC_CACHE = build_kernel()
    return _NC_CACHE


def kernel(**inputs: np.ndarray) -> np.ndarray:
    from concourse.bass_utils import run_bass_kernel_spmd

    x = np.ascontiguousarray(inputs["x"], dtype=np.float32)
    cb = np.ascontiguousarray(inputs["codebook"], dtype=np.float32)
    assert x.shape == (128, 16, 64, 64) and cb.shape == (K, D)

    nc = _get_nc()
    in_maps = [
        {"x": x[i * NI:(i + 1) * NI], "codebook": cb} for i in range(N_CORES)
    ]
    res = run_bass_kernel_spmd(nc, in_maps, core_ids=list(range(N_CORES)))
    return np.concatenate([res.results[i]["out"] for i in range(N_CORES)], axis=0)


# revision 12
# speedup vs baseline: 1.0573x; 1.0573x over previous
"""Trainium2 Bass kernel for nn_Codebook (vq_codebook).

Pipeline per core (16 of 128 images, data-parallel over 8 cores):
  1. Precompute (on device, once per core):
     - CBT_k [128, 1024]: codebook transposed into (c,ph)x{pw,pw+1} pair layout
     - cbsq broadcast [128, 1024]
     - T[b, :] = sum_j gauss(b - j) * codebook[j, :]  (banded Toeplitz matmul)
       stored to DRAM scratch for row-gather.
  2. Per image: load x twice into SBUF partitions 64-127 (offset 0) and
     0-63 (shifted +1 element) so a single strided AP provides the
     transposed patch matrix with full 128-deep contraction.
  3. scores m = (2*G - rowsq) - cbsq  (exact negation of the reference's
     f32-rounded d2, so argmax matches jnp.argmin bit-for-bit).
  4. DVE max/max_index -> bmu; indirect DMA gathers T[bmu] rows.
  5. Unpatchify via 4 PE transposes per row-tile + strided ACT copies;
     contiguous 256B-run DMA out.
"""
import math
import sys

if "/opt/trn_rl_repo" not in sys.path:
    sys.path.insert(0, "/opt/trn_rl_repo")

import numpy as np

import concourse.bass as bass
import concourse.mybir as mybir
from concourse.bass import IndirectOffsetOnAxis
from concourse.masks import make_identity
from concourse.tile import TileContext

# ---------------------------------------------------------------------------
# Patch: this walrus build rejects >1 sem wait on the tail Drain; spread the
# waits across single-wait SP nops instead.
import concourse.tile as _tile_mod
from concourse.vector_clock import ScopedClock as _ScopedClock


def _patched_drain_and_barrier(self, tick_clock, wait_clock):
    nc = self.nc
    drain_inst = nc.sync.drain()
    wait_clock.add_sem_waits(
        drain_inst.ins, _ScopedClock({None: tick_clock.global_clock})
    )
    si = drain_inst.ins.sync_info
    waits = list(si.on_wait) if si is not None else []
    if len(waits) > 1:
        si.on_wait = waits[:1]
        for w in waits[1:]:
            nop = nc.sync.nop(nofuse=True)
            nop.ins.sync_info = mybir.SyncInfo(on_wait=[w], on_update=[])

    nc.all_engine_barrier()
    assert self.sems is not None
    popped = nc._tile_sem_poison_stack.pop()
    assert popped is self._sem_poison
    nc.clear_and_free_semaphores(list(self.sems.allocated().values()))
    nc.all_engine_barrier()


_tile_mod.TileContext._drain_and_barrier = _patched_drain_and_barrier

# Generalized: any instruction may carry at most MAX_WAITS sem waits in this
# walrus build. Hoist extras onto same-engine NoOps committed just before
# (engines execute in order, so an earlier wait is equivalent).
MAX_WAITS = 1
_orig_commit = _tile_mod.TileContext._commit_instruction
_waitsplit_id = [0]


def _patched_commit(self, inst, lazy_reg_writes=True):
    si = inst.sync_info
    if si is not None:
        waits = list(si.on_wait)
        if len(waits) > MAX_WAITS:
            keep = waits[:MAX_WAITS - 1] if MAX_WAITS > 1 else []
            extra = waits[len(keep):]
            si.on_wait = keep + extra[-1:]
            extra = extra[:-1]
            for i in range(0, len(extra), MAX_WAITS):
                _waitsplit_id[0] += 1
                nop = mybir.InstNoOp(
                    name=f"I-waitsplit-{_waitsplit_id[0]}", ins=[], outs=[]
                )
                nop.engine = inst.engine
                nop.sync_info = mybir.SyncInfo(
                    on_wait=extra[i:i + MAX_WAITS], on_update=[]
                )
                self._add_instruction(nop)
    return _orig_commit(self, inst, lazy_reg_writes)


_tile_mod.TileContext._commit_instruction = _patched_commit
# ---------------------------------------------------------------------------

F32 = mybir.dt.float32
U32 = mybir.dt.uint32
AX = mybir.AxisListType
ALU = mybir.AluOpType
ACTF = mybir.ActivationFunctionType

N_CORES = 8
NI = 16          # images per core
K = 1024         # codebook entries
D = 256          # embedding dim
NEG_INV_2VAR = -1.0 / (2.0 * (-(256.0 / (2.0 * math.log(0.1)))))


def build_kernel():
    nc = bass.Bass()
    x = nc.dram_tensor("x", [NI, 16, 64, 64], F32, kind="ExternalInput")
    cb = nc.dram_tensor("codebook", [K, D], F32, kind="ExternalInput")
    out = nc.dram_tensor("out", [NI, 16, 64, 64], F32, kind="ExternalOutput")
    t_dram = nc.dram_tensor("t_scratch", [K, D], F32, kind="Internal")

    with TileContext(nc) as tc:
        with (
            tc.tile_pool(name="const", bufs=1) as cpool,
            tc.tile_pool(name="x2", bufs=3) as xpool,
            tc.tile_pool(name="scores", bufs=4) as spool,
            tc.tile_pool(name="small", bufs=8) as smpool,
            tc.tile_pool(name="q", bufs=4) as qpool,
            tc.tile_pool(name="outsb", bufs=3) as opool,
            tc.tile_pool(name="pg", bufs=2, space="PSUM") as pg,
            tc.tile_pool(name="pgr", bufs=2, space="PSUM") as pgr,
            tc.tile_pool(name="pu", bufs=2, space="PSUM") as pu,
        ):
            # ---------------- constants / precompute ----------------
            ident = cpool.tile([128, 128], F32)
            make_identity(nc, ident[:])
            ones_col = cpool.tile([128, 1], F32)
            nc.vector.memset(ones_col[:], 1.0)
            ones_row = cpool.tile([1, 128], F32)
            nc.vector.memset(ones_row[:], 1.0)

            # W band table [128, 384]: W[p, u] = gauss(u - 128 - p)
            wtab = cpool.tile([128, 384], F32)
            nc.gpsimd.iota(
                wtab[:], pattern=[[1, 384]], base=-128, channel_multiplier=-1,
                allow_small_or_imprecise_dtypes=True,
            )
            wsq = cpool.tile([128, 384], F32)
            nc.scalar.activation(wsq[:], wtab[:], ACTF.Square)
            nc.scalar.activation(wtab[:], wsq[:], ACTF.Exp, scale=NEG_INV_2VAR)

            # codebook natural chunks: cb_all[p, jc*256+d] = cb[jc*128+p, d]
            cb_all = cpool.tile([128, 8 * D], F32)
            nc.sync.dma_start(
                out=cb_all[:].rearrange("p (jc d) -> p jc d", jc=8),
                in_=cb[:].rearrange("(jc p) d -> p jc d", p=128),
            )

            # CBT_k [128, 1024]: p<64 -> cb[j, 4a+k]; p>=64 -> cb[j, 4a+k+1]
            cbt0 = cpool.tile([128, K], F32, tag="cbt0")
            cbt2 = cpool.tile([128, K], F32, tag="cbt2")
            cbt = {0: cbt0, 2: cbt2}
            stage0 = cpool.tile([64, K], F32, tag="stage0")
            stage2 = cpool.tile([64, K], F32, tag="stage2")
            stage = {0: stage0, 2: stage2}
            cb_r = cb_all[:].rearrange("p (jc a pw) -> p jc a pw", jc=8, pw=4)
            for jc in range(8):
                for pw in range(4):
                    k = 0 if pw < 2 else 2
                    # pw even -> cbt_k lower half; pw odd -> staged, DMA-shifted
                    tgt = cbt[k] if pw % 2 == 0 else stage[k]
                    tp = pu.tile([64, 128], F32, tag="pu")
                    # in_: [128 j, 64 a] strided slice (offset pw, stride 4)
                    src = cb_r[:, jc, :, pw]
                    nc.tensor.transpose(out=tp[:], in_=src, identity=ident[:])
                    nc.scalar.activation(
                        tgt[0:64, jc * 128:(jc + 1) * 128], tp[:], ACTF.Copy
                    )
            for k in (0, 2):
                nc.sync.dma_start(out=cbt[k][64:128, :], in_=stage[k][0:64, :])

            # cbsq row then broadcast [128, 1024]
            cbsq_bcast = cpool.tile([128, K], F32)
            sqtmp = spool.tile([128, K], F32, tag="scores")
            pb = pgr.tile([1, 512], F32, tag="pgr")
            pb2 = pgr.tile([1, 512], F32, tag="pgr")
            pbs = (pb, pb2)  # noqa
            for ki, k in enumerate((0, 2)):
                nc.vector.tensor_tensor(sqtmp[:], cbt[k][:], cbt[k][:], ALU.mult)
                for h in range(2):
                    nc.tensor.matmul(
                        pbs[h][:], ones_col[:], sqtmp[:, h * 512:(h + 1) * 512],
                        start=(ki == 0), stop=(ki == 1),
                    )
            cbsq_row = smpool.tile([1, K], F32, tag="cbsqrow")
            for h in range(2):
                nc.scalar.activation(
                    cbsq_row[0:1, h * 512:(h + 1) * 512], pbs[h][:], ACTF.Copy
                )
            pc = pg.tile([128, 1024], F32, tag="pg")
            for h in range(2):
                nc.tensor.matmul(
                    pc[:, h * 512:(h + 1) * 512], ones_row[:],
                    cbsq_row[0:1, h * 512:(h + 1) * 512],
                    start=True, stop=True,
                )
            nc.scalar.activation(cbsq_bcast[:], pc[:], ACTF.Copy)

            # T table: T[bc*128+p, :] = sum_j gauss(b-j) cb[j, :]
            t_write_insts = []
            for bc in range(8):
                pt = pg.tile([128, 1024], F32, tag="pg")
                deltas = [d_ for d_ in (-1, 0, 1) if 0 <= bc + d_ < 8]
                for i, d_ in enumerate(deltas):
                    off = 128 * (1 - d_)
                    jc = bc + d_
                    nc.tensor.matmul(
                        pt[:, 0:D],
                        wtab[:, off:off + 128],
                        cb_all[:, jc * D:(jc + 1) * D],
                        start=(i == 0), stop=(i == len(deltas) - 1),
                    )
                t_sb = qpool.tile([128, D], F32, tag="q")
                nc.scalar.activation(t_sb[:], pt[:, 0:D], ACTF.Copy)
                wi = nc.sync.dma_start(
                    out=t_dram[bc * 128:(bc + 1) * 128, :], in_=t_sb[:]
                )
                t_write_insts.append(wi.ins)

            # ---------------- main loop over images ----------------
            for n in range(NI):
                x2 = xpool.tile([128, 1028], F32)
                for ph in range(4):
                    # partitions 64 + 4c + ph  <-  x[n, :, ph::4, :]
                    nc.sync.dma_start(
                        out=x2[64 + ph:128:4, 0:1024],
                        in_=x[n][:, ph::4, :],
                    )
                nc.sync.dma_start(out=x2[0:64, 1:1025], in_=x2[64:128, 0:1024])

                out_sb = opool.tile([64, 1024], F32)

                for t in range(2):
                    pgt = pg.tile([128, 1024], F32, tag="pg")
                    pgram = pgr.tile([128, 128], F32, tag="pgr")
                    for ki, k in enumerate((0, 2)):
                        o = 1 + k + 512 * t
                        lhsT = x2[:, o:o + 512].rearrange(
                            "p (hp wp pw) -> p hp wp pw", wp=16, pw=4
                        )[:, :, :, 0]
                        st, sp = (ki == 0), (ki == 1)
                        nc.tensor.matmul(pgt[:, 0:512], lhsT, cbt[k][:, 0:512],
                                         start=st, stop=sp)
                        nc.tensor.matmul(pgt[:, 512:1024], lhsT,
                                         cbt[k][:, 512:1024],
                                         start=st, stop=sp)
                        nc.tensor.matmul(pgram[:], lhsT, lhsT,
                                         start=st, stop=sp)
                    # rowsq = diag(gram)
                    diag = smpool.tile([128, 128], F32, tag="diag")
                    nc.vector.tensor_tensor(diag[:], pgram[:], ident[:], ALU.mult)
                    rowsq = smpool.tile([128, 1], F32, tag="rowsq")
                    nc.vector.tensor_reduce(rowsq[:], diag[:], AX.X, ALU.add)
                    nrowsq = smpool.tile([128, 1], F32, tag="nrowsq")
                    nc.vector.tensor_scalar(
                        nrowsq[:], rowsq[:], -1.0, None, op0=ALU.mult
                    )
                    # m = (2G - rowsq) - cbsq  == -(reference d2), same rounding
                    sc = spool.tile([128, K], F32, tag="scores")
                    nc.scalar.activation(
                        sc[:], pgt[:], ACTF.Identity,
                        bias=nrowsq[:, 0:1], scale=2.0,
                    )
                    nc.vector.tensor_tensor(sc[:], sc[:], cbsq_bcast[:], ALU.subtract)
                    mx8 = smpool.tile([128, 8], F32, tag="mx8")
                    idx8 = smpool.tile([128, 8], U32, tag="idx8")
                    nc.vector.max(mx8[:], sc[:])
                    nc.vector.max_index(idx8[:], mx8[:], sc[:])

                    q = qpool.tile([128, D], F32, tag="q")
                    gi = nc.gpsimd.indirect_dma_start(
                        out=q[:],
                        out_offset=None,
                        in_=t_dram[:],
                        in_offset=IndirectOffsetOnAxis(ap=idx8[:, 0:1], axis=0),
                    )
                    for twi in t_write_insts:
                        _tile_mod.add_dep_helper(
                            gi.ins, twi, reason="gather waits for T table"
                        )

                    # unpatchify: 4 pw-slice transposes into one psum bank,
                    # then a single strided ACT copy
                    q_r = q[:].rearrange("p (a pw) -> p a pw", pw=4)
                    put = pu.tile([64, 512], F32, tag="pu")
                    for pw in range(4):
                        nc.tensor.transpose(
                            out=put[:, pw * 128:(pw + 1) * 128],
                            in_=q_r[:, :, pw], identity=ident[:],
                        )
                    # put free = (pw, hp_lo, wp); dest free = (hp, wp, pw)
                    o_r = out_sb[:].rearrange(
                        "p (hp wp pw) -> p pw hp wp", wp=16, pw=4
                    )
                    dst = o_r[0:64, :, 8 * t:8 * (t + 1), :]
                    nc.scalar.activation(dst, put[:], ACTF.Copy)

                for ph in range(4):
                    nc.sync.dma_start(
                        out=out[n][:, ph::4, :],
                        in_=out_sb[ph:64:4, :],
                    )

    return nc


_NC_CACHE = None


def _get_nc():
    global _NC_CACHE
    if _NC_CACHE is None:
        _NC_CACHE = build_kernel()
    return _NC_CACHE


def kernel(**inputs: np.ndarray) -> np.ndarray:
    from concourse.bass_utils import run_bass_kernel_spmd

    x = np.ascontiguousarray(inputs["x"], dtype=np.float32)
    cb = np.ascontiguousarray(inputs["codebook"], dtype=np.float32)
    assert x.shape == (128, 16, 64, 64) and cb.shape == (K, D)

    nc = _get_nc()
    in_maps = [
        {"x": x[i * NI:(i + 1) * NI], "codebook": cb} for i in range(N_CORES)
    ]
    res = run_bass_kernel_spmd(nc, in_maps, core_ids=list(range(N_CORES)))
    return np.concatenate([res.results[i]["out"] for i in range(N_CORES)], axis=0)
